# revision 7
# baseline (speedup 1.0000x reference)
"""Trainium2 Bass kernel for nn_DeformedAgent (sparse_attention).

Self-contained: kernel(**inputs) -> np.ndarray [128, 64, 64].

Math (validated against the neuron-executed reference):
  - The deformable gather index is idx[b,k] = clip(k + (offset_raw[b,k] > 0), <=63)
    because the device float->int cast rounds to nearest (ref points sit at
    k+0.5, tanh-bounded offsets |d|<1/63 never cross the next integer).
  - offset_raw = conv1d(w_off) o gelu o channelLN o depthwise-conv9(qT)+b.
    Only its SIGN is consumed, but it must be f32-accurate (min margin 3.5e-3).
  - The align_corners interp is an exact subsample t_s = tokens'[:, ::65]
    (src positions k*65 are integral), and the torch-style raw reshapes make
    the output out[b] = x_s @ wq.T + bq with x_s flat-aliasing t_s.

Per-core (8 cores, data-parallel over batch):
  phase 1: depthwise conv as 9 diag-matmuls + K=1 bias matmul (f32r),
           channel-LN via ones-matmul partition sums, ACT gelu,
           offset contraction on DVE + window reduce.
  phase 2: build one-hot gather matrix from sign(offset), apply as matmul,
           proj matmul (f32) + bias via K=1 matmul.
"""
import numpy as np

import concourse.bass as bass
import concourse.mybir as mybir
import concourse.tile as tile
from concourse import bacc
from concourse.bass_utils import run_bass_kernel_spmd

F32 = mybir.dt.float32
F32R = mybir.dt.float32r
AF = mybir.ActivationFunctionType
ALU = mybir.AluOpType
AX = mybir.AxisListType

B, L, C, CLS, LK, KK, AGENT = 8, 4096, 1024, 1, 64, 9, 64
EPS = 1e-5
LH = L // 2          # l per half = 2048
NLC = LH // 512      # 512-chunks per half = 4

# ---- packF (f32 consts) column layout, [128, NF] ----
PF_BQ = 0        # [1,1024] row0
PF_KV = 1024     # [1,64] row0: k+0.5
PF_EPS = 1088    # [1,1] row0
PF_ONE = 1096    # [1,128] row0 ones  (K=1 lhsT / bcast lhsT)
PF_ONEC = 1224   # [128,1] ones column (partition-sum lhsT)
PF_LNW = 1225    # [128,8]
PF_LNB = 1233    # [128,8]
PF_W2 = 1241     # [128, 8*64] w_off arranged
PF_G2 = 1753     # [64,128] rows0:64: G2[kk',p]=kk'
NF = 1881

# ---- packR (f32r consts) ----
PR_ONE = 0       # [1,512] row0 ones
PR_BDW = 512     # [1,1024] row0 b_dw
NR = 1536


def _build(nc):
    qT = nc.dram_tensor("qT", [C, L + 8], F32R, kind="ExternalInput")
    dW = nc.dram_tensor("dW", [128, 8 * KK * 128], F32R, kind="ExternalInput")
    pR = nc.dram_tensor("pR", [1, NR], F32R, kind="ExternalInput")
    pF = nc.dram_tensor("pF", [128, NF], F32, kind="ExternalInput")
    tsT = nc.dram_tensor("tsT", [64, 1024], F32, kind="ExternalInput")  # even|odd
    wqT = nc.dram_tensor("wqT", [C, C], F32, kind="ExternalInput")
    out = nc.dram_tensor("out", [64, C], F32, kind="ExternalOutput")

    with tile.TileContext(nc) as tc:
        with (
            tc.tile_pool(name="consts", bufs=1) as cpool,
            tc.tile_pool(name="strips", bufs=2) as spool,
            tc.tile_pool(name="dwpool", bufs=2) as dwpool,
            tc.tile_pool(name="ybuf", bufs=1) as ypool,
            tc.tile_pool(name="acc", bufs=1) as apool,
            tc.tile_pool(name="scratch", bufs=2) as scr,
            tc.tile_pool(name="ysqp", bufs=1) as ysqp,
            tc.tile_pool(name="rows", bufs=1) as rows,
            tc.tile_pool(name="wq", bufs=2) as wqpool,
            tc.tile_pool(name="pconv", bufs=1, space="PSUM") as pconv,
            tc.tile_pool(name="pstat", bufs=1, space="PSUM") as pstat,
            tc.tile_pool(name="pbc", bufs=1, space="PSUM") as pbc,
        ):
            pR_t = cpool.tile([1, NR], F32R, tag="pR")
            nc.sync.dma_start(pR_t[:], pR[:, :])
            pF_t = cpool.tile([128, NF], F32, tag="pF")
            nc.sync.dma_start(pF_t[:], pF[:, :])
            tsT_t = cpool.tile([64, 1024], F32, tag="tsT")
            nc.sync.dma_start(tsT_t[:], tsT[:, :])

            onesR = pR_t[0:1, PR_ONE:PR_ONE + 512]
            bdw_row = pR_t[0:1, PR_BDW:PR_BDW + C]
            oneF = pF_t[0:1, PF_ONE:PF_ONE + 128].bitcast(F32)
            oneC = pF_t[:, PF_ONEC:PF_ONEC + 1].bitcast(F32)
            eps_ap = pF_t[0:1, PF_EPS:PF_EPS + 1].bitcast(F32)

            # offset accumulator [128, 64] f32
            offA = apool.tile([128, LK], F32, tag="offA")
            nc.vector.memset(offA[:], 0.0)

            y_t = [ypool.tile([128, LH], F32, tag=f"y{cc}", name=f"y{cc}")
                   for cc in range(8)]

            for h in range(2):
                T = apool.tile([128, LH], F32, tag="T")
                Tsq = apool.tile([128, LH], F32, tag="Tsq")
                nc.vector.memset(T[:], 0.0)
                nc.vector.memset(Tsq[:], 0.0)

                # ---- conv for all cc of this half ----
                for cc in range(8):
                    strip = spool.tile([128, LH + 8], F32R, tag="strip")
                    nc.sync.dma_start(
                        strip[:], qT[128 * cc:128 * (cc + 1),
                                     LH * h:LH * h + LH + 8])
                    dW_t = dwpool.tile([128, KK * 128], F32R, tag="dWcc")
                    nc.sync.dma_start(
                        dW_t[:], dW[:, KK * 128 * cc:KK * 128 * (cc + 1)])
                    ps = [pconv.tile([128, 512], F32, tag=f"pc{lc}", name=f"pc{lc}")
                          for lc in range(NLC)]
                    for j in range(KK):
                        wslice = dW_t[:, 128 * j:128 * (j + 1)]
                        for lc in range(NLC):
                            nc.tensor.matmul(
                                ps[lc][:], wslice,
                                strip[:, 512 * lc + j:512 * lc + j + 512],
                                start=(j == 0), stop=False)
                    for lc in range(NLC):
                        nc.tensor.matmul(
                            ps[lc][:], bdw_row[:, 128 * cc:128 * (cc + 1)],
                            onesR, start=False, stop=True)
                    # evict + accumulate stats inputs
                    for lc in range(NLC):
                        nc.scalar.copy(y_t[cc][:, 512 * lc:512 * (lc + 1)],
                                       ps[lc][:])
                    ysq = ysqp.tile([128, LH], F32, tag="ysq")
                    nc.scalar.square(ysq[:], y_t[cc][:])
                    nc.vector.tensor_tensor(T[:], T[:], y_t[cc][:], ALU.add)
                    nc.vector.tensor_tensor(Tsq[:], Tsq[:], ysq[:], ALU.add)

                # ---- stats + normalize + gelu + offset, per 512-chunk ----
                for lc in range(NLC):
                    sl = slice(512 * lc, 512 * (lc + 1))
                    s1 = pstat.tile([1, 512], F32, tag="s1")
                    s2 = pstat.tile([1, 512], F32, tag="s2")
                    nc.tensor.matmul(s1[:], oneC, T[:, sl].bitcast(F32),
                                     start=True, stop=True)
                    nc.tensor.matmul(s2[:], oneC, Tsq[:, sl].bitcast(F32),
                                     start=True, stop=True)
                    mu = rows.tile([1, 512], F32, tag="mu")
                    nc.vector.tensor_scalar(mu[:], s1[:], 1.0 / C, None, ALU.mult)
                    e2 = rows.tile([1, 512], F32, tag="e2")
                    nc.vector.tensor_scalar(e2[:], s2[:], 1.0 / C, None, ALU.mult)
                    var = rows.tile([1, 512], F32, tag="var")
                    nc.vector.tensor_tensor(var[:], mu[:], mu[:], ALU.mult)
                    nc.vector.tensor_tensor(var[:], e2[:], var[:], ALU.subtract)
                    sd = rows.tile([1, 512], F32, tag="sd")
                    nc.scalar.activation(sd[:], var[:], AF.Sqrt, bias=eps_ap)
                    rstd = rows.tile([1, 512], F32, tag="rstd")
                    nc.vector.reciprocal(rstd[:], sd[:])
                    brow = rows.tile([1, 512], F32, tag="brow")
                    nc.vector.tensor_tensor(brow[:], mu[:], rstd[:], ALU.mult)
                    nc.vector.tensor_scalar(brow[:], brow[:], -1.0, None, ALU.mult)
                    # broadcast rows to [128, 512] via K=1 f32 matmuls
                    A_ps = pbc.tile([128, 512], F32, tag="A")
                    nc.tensor.matmul(A_ps[:], oneF, rstd[:], start=True, stop=True)
                    B_ps = pbc.tile([128, 512], F32, tag="Bb")
                    nc.tensor.matmul(B_ps[:], oneF, brow[:], start=True, stop=True)

                    for cc in range(8):
                        z = scr.tile([128, 512], F32, tag="z")
                        nc.vector.tensor_tensor(z[:], y_t[cc][:, sl], A_ps[:],
                                                ALU.mult)
                        nc.vector.tensor_tensor(z[:], z[:], B_ps[:], ALU.add)
                        nc.vector.tensor_scalar(
                            z[:], z[:],
                            pF_t[:, PF_LNW + cc:PF_LNW + cc + 1].bitcast(F32),
                            pF_t[:, PF_LNB + cc:PF_LNB + cc + 1].bitcast(F32),
                            ALU.mult, ALU.add)
                        g = scr.tile([128, 512], F32, tag="g")
                        nc.scalar.activation(g[:], z[:], AF.Gelu)
                        om = scr.tile([128, 512], F32, tag="om")
                        w2 = pF_t[:, PF_W2 + 64 * cc:PF_W2 + 64 * cc + 64]
                        nc.vector.tensor_tensor(
                            om[:].rearrange("p (k j) -> p k j", j=64),
                            g[:].rearrange("p (k j) -> p k j", j=64),
                            w2.bitcast(F32)[:, None, :].to_broadcast((128, 8, 64)),
                            ALU.mult)
                        red = scr.tile([128, 8], F32, tag="red")
                        nc.vector.tensor_reduce(
                            red[:], om[:].rearrange("p (k j) -> p k j", j=64),
                            AX.X, ALU.add)
                        ko = 32 * h + 8 * lc
                        nc.vector.tensor_tensor(offA[:, ko:ko + 8],
                                                offA[:, ko:ko + 8], red[:],
                                                ALU.add)

            # ---- offset signs -> one-hot gather matrix P2 [64, 128] ----
            offp = pstat.tile([1, 64], F32, tag="s1")
            nc.tensor.matmul(offp[:], oneC, offA[:].bitcast(F32),
                             start=True, stop=True)
            sgn = rows.tile([1, 64], F32, tag="sgn")
            nc.scalar.sign(sgn[:], offp[:])
            posd = rows.tile([1, 128], F32, tag="posd")
            # pos = k + 0.5 + 0.4*sign(off); duplicate into both 64-halves
            for lsb in range(2):
                nc.vector.scalar_tensor_tensor(
                    posd[:, 64 * lsb:64 * lsb + 64], sgn[:], 0.4,
                    pF_t[0:1, PF_KV:PF_KV + 64].bitcast(F32),
                    ALU.mult, ALU.add)
            nc.vector.tensor_scalar(posd[:], posd[:], 63.49, None, ALU.min)
            pos_ps = pbc.tile([64, 128], F32, tag="A")
            nc.tensor.matmul(pos_ps[:], oneF[:, 0:64], posd[:],
                             start=True, stop=True)
            dmat = scr.tile([64, 128], F32, tag="dmat")
            nc.vector.tensor_tensor(
                dmat[:], pF_t[0:64, PF_G2:PF_G2 + 128].bitcast(F32),
                pos_ps[:], ALU.subtract)
            nc.scalar.activation(dmat[:], dmat[:], AF.Abs)
            P2 = scr.tile([64, 128], F32, tag="P2")
            nc.vector.tensor_scalar(P2[:], dmat[:], 0.5, None, ALU.is_lt)

            # ---- gather: T2[64*lsb + k, u] = t_s[2u + lsb, idx[k]] ----
            T2_ps = pbc.tile([128, 512], F32, tag="Bb")
            nc.tensor.matmul(T2_ps[0:64, :], P2[:, 0:64],
                             tsT_t[:, 0:512], start=True, stop=True)
            nc.tensor.matmul(T2_ps[64:128, :], P2[:, 64:128],
                             tsT_t[:, 512:1024], start=True, stop=True,
                             tile_position=(0, 64))
            T2 = scr.tile([128, 512], F32, tag="T2")
            nc.scalar.copy(T2[:], T2_ps[:])

            # ---- proj: out[n,o] = sum_m x_sT[m,n] wqT[m,o] + bq[o] ----
            out_sb = scr.tile([64, C], F32, tag="outsb")
            wqr = wqT.rearrange("(t p) o -> p t o", p=128)
            for oh in range(2):
                wq_t = wqpool.tile([128, 8, 512], F32, tag="wqh")
                nc.sync.dma_start(wq_t[:], wqr[:, :, 512 * oh:512 * (oh + 1)])
                po = pconv.tile([64, 512], F32, tag="pc0")
                for t in range(8):
                    nc.tensor.matmul(po[:], T2[:, t::8],
                                     wq_t[:, t, :],
                                     start=(t == 0), stop=False)
                nc.tensor.matmul(po[:], oneF[:, 0:64],
                                 pF_t[0:1, PF_BQ + 512 * oh:PF_BQ + 512 * (oh + 1)]
                                 .bitcast(F32),
                                 start=False, stop=True)
                nc.scalar.copy(out_sb[:, 512 * oh:512 * (oh + 1)], po[:])
            nc.sync.dma_start(out[:, :], out_sb[:])

    nc.finalize()
    return nc


_NC_CACHE = {}


def _get_nc():
    if "nc" not in _NC_CACHE:
        nc = bacc.Bacc("TRN2", target_bir_lowering=False, debug=False,
                       num_devices=8)
        _NC_CACHE["nc"] = _build(nc)
    return _NC_CACHE["nc"]


def _host_prep(q, tokens, w_dw, b_dw, ln_w, ln_b, w_off, wq, bq):
    """Build per-core input maps."""
    q = np.ascontiguousarray(q, dtype=np.float32)
    tokens = np.ascontiguousarray(tokens, dtype=np.float32)

    # shared consts
    dW = np.zeros((128, 8 * KK * 128), dtype=np.float32)
    ar = np.arange(128)
    for ccx in range(8):
        for j in range(KK):
            dW[ar, KK * 128 * ccx + 128 * j + ar] = w_dw[128 * ccx + ar, 0, j]

    pR = np.zeros((1, NR), dtype=np.float32)
    pR[0, PR_ONE:PR_ONE + 512] = 1.0
    pR[0, PR_BDW:PR_BDW + C] = b_dw

    pF = np.zeros((128, NF), dtype=np.float32)
    pF[0, PF_BQ:PF_BQ + C] = bq
    pF[0, PF_KV:PF_KV + 64] = np.arange(64) + 0.5
    pF[0, PF_EPS] = EPS
    pF[0, PF_ONE:PF_ONE + 128] = 1.0
    pF[:, PF_ONEC] = 1.0
    pF[:, PF_LNW:PF_LNW + 8] = ln_w.reshape(8, 128).T
    pF[:, PF_LNB:PF_LNB + 8] = ln_b.reshape(8, 128).T
    pF[:, PF_W2:PF_W2 + 512] = w_off[0].reshape(8, 128, 64).transpose(
        1, 0, 2).reshape(128, 512)
    pF[0:64, PF_G2:PF_G2 + 128] = np.tile(
        np.arange(64, dtype=np.float32)[:, None], (1, 128))

    wqT = np.ascontiguousarray(wq.T)

    in_maps = []
    for b in range(B):
        qTb = np.zeros((C, L + 8), dtype=np.float32)
        qTb[:, 4:4 + L] = q[b, CLS:, :].T
        flat = tokens[b].reshape(-1)[C:]
        t_s = flat.reshape(C, L)[:, ::65]            # [C, 64]
        tsT = np.empty((64, 1024), dtype=np.float32)
        tsT[:, 0:512] = t_s[0::2, :].T               # even c
        tsT[:, 512:1024] = t_s[1::2, :].T            # odd c
        in_maps.append(dict(qT=qTb, dW=dW, pR=pR, pF=pF,
                            tsT=np.ascontiguousarray(tsT), wqT=wqT))
    return in_maps


def kernel(q, tokens, w_dw, b_dw, ln_w, ln_b, w_off, wq, bq, _trace=False):
    nc = _get_nc()
    in_maps = _host_prep(np.asarray(q), np.asarray(tokens), np.asarray(w_dw),
                         np.asarray(b_dw), np.asarray(ln_w), np.asarray(ln_b),
                         np.asarray(w_off), np.asarray(wq), np.asarray(bq))
    last_err = None
    for attempt in range(3):
        try:
            res = run_bass_kernel_spmd(nc, in_maps, core_ids=list(range(B)),
                                       trace=_trace)
            break
        except Exception as e:  # transient NRT/device hiccups: retry
            last_err = e
            import time as _time
            _time.sleep(2.0)
    else:
        raise last_err
    outs = np.stack([res.results[b]["out"] for b in range(B)])  # [8, 64, 1024]
    if _trace:
        kernel._last_result = res
    return outs.reshape(B * 16, 64, LK).astype(np.float32)


# revision 8
# speedup vs baseline: 1.0188x; 1.0188x over previous
"""Trainium2 Bass kernel for nn_DeformedAgent (sparse_attention).

Self-contained: kernel(**inputs) -> np.ndarray [128, 64, 64].

Math (validated against the neuron-executed reference):
  - The deformable gather index is idx[b,k] = clip(k + (offset_raw[b,k] > 0), <=63)
    because the device float->int cast rounds to nearest (ref points sit at
    k+0.5, tanh-bounded offsets |d|<1/63 never cross the next integer).
  - offset_raw = conv1d(w_off) o gelu o channelLN o depthwise-conv9(qT)+b.
    Only its SIGN is consumed, but it must be f32-accurate (min margin 3.5e-3).
  - The align_corners interp is an exact subsample t_s = tokens'[:, ::65]
    (src positions k*65 are integral), and the torch-style raw reshapes make
    the output out[b] = x_s @ wq.T + bq with x_s flat-aliasing t_s.

Per-core (8 cores, data-parallel over batch):
  phase 1: depthwise conv as 9 diag-matmuls + K=1 bias matmul (f32r),
           channel-LN via ones-matmul partition sums, ACT gelu,
           offset contraction on DVE + window reduce.
  phase 2: build one-hot gather matrix from sign(offset), apply as matmul,
           proj matmul (f32) + bias via K=1 matmul.
"""
import numpy as np

import concourse.bass as bass
import concourse.mybir as mybir
import concourse.tile as tile
from concourse import bacc
from concourse.bass_utils import run_bass_kernel_spmd

F32 = mybir.dt.float32
F32R = mybir.dt.float32r
AF = mybir.ActivationFunctionType
ALU = mybir.AluOpType
AX = mybir.AxisListType

B, L, C, CLS, LK, KK, AGENT = 8, 4096, 1024, 1, 64, 9, 64
EPS = 1e-5
LH = L // 2          # l per half = 2048
NLC = LH // 512      # 512-chunks per half = 4

# ---- packF (f32 consts) column layout, [128, NF] ----
PF_BQ = 0        # [1,1024] row0
PF_KV = 1024     # [1,64] row0: k+0.5
PF_EPS = 1088    # [1,1] row0
PF_ONE = 1096    # [1,128] row0 ones  (K=1 lhsT / bcast lhsT)
PF_ONEC = 1224   # [128,1] ones column (partition-sum lhsT)
PF_LNW = 1225    # [128,8]
PF_LNB = 1233    # [128,8]
PF_W2 = 1241     # [128, 8*64] w_off arranged
PF_G2 = 1753     # [64,128] rows0:64: G2[kk',p]=kk'
NF = 1881

# ---- packR (f32r consts) ----
PR_ONE = 0       # [1,512] row0 ones
PR_BDW = 512     # [1,1024] row0 b_dw
NR = 1536


def _build(nc):
    qT = nc.dram_tensor("qT", [C, L + 8], F32R, kind="ExternalInput")
    dW = nc.dram_tensor("dW", [128, 8 * KK * 128], F32R, kind="ExternalInput")
    pR = nc.dram_tensor("pR", [1, NR], F32R, kind="ExternalInput")
    pF = nc.dram_tensor("pF", [128, NF], F32, kind="ExternalInput")
    tsT = nc.dram_tensor("tsT", [64, 1024], F32, kind="ExternalInput")  # even|odd
    wqT = nc.dram_tensor("wqT", [C, C], F32, kind="ExternalInput")
    out = nc.dram_tensor("out", [64, C], F32, kind="ExternalOutput")

    with tile.TileContext(nc) as tc:
        with (
            tc.tile_pool(name="consts", bufs=1) as cpool,
            tc.tile_pool(name="strips", bufs=2) as spool,
            tc.tile_pool(name="dwpool", bufs=2) as dwpool,
            tc.tile_pool(name="ybuf", bufs=1) as ypool,
            tc.tile_pool(name="acc", bufs=1) as apool,
            tc.tile_pool(name="flow", bufs=4) as flow,
            tc.tile_pool(name="scratch", bufs=1) as scr,
            tc.tile_pool(name="ysqp", bufs=1) as ysqp,
            tc.tile_pool(name="rows", bufs=1) as rows,
            tc.tile_pool(name="wq", bufs=2) as wqpool,
            tc.tile_pool(name="pconv", bufs=1, space="PSUM") as pconv,
            tc.tile_pool(name="pstat", bufs=1, space="PSUM") as pstat,
            tc.tile_pool(name="pbc", bufs=1, space="PSUM") as pbc,
        ):
            pR_t = cpool.tile([1, NR], F32R, tag="pR")
            nc.sync.dma_start(pR_t[:], pR[:, :])
            pF_t = cpool.tile([128, NF], F32, tag="pF")
            nc.sync.dma_start(pF_t[:], pF[:, :])
            tsT_t = cpool.tile([64, 1024], F32, tag="tsT")
            nc.sync.dma_start(tsT_t[:], tsT[:, :])

            onesR = pR_t[0:1, PR_ONE:PR_ONE + 512]
            bdw_row = pR_t[0:1, PR_BDW:PR_BDW + C]
            oneF = pF_t[0:1, PF_ONE:PF_ONE + 128].bitcast(F32)
            oneC = pF_t[:, PF_ONEC:PF_ONEC + 1].bitcast(F32)
            eps_ap = pF_t[0:1, PF_EPS:PF_EPS + 1].bitcast(F32)

            # offset accumulator [128, 64] f32
            offA = apool.tile([128, LK], F32, tag="offA")
            nc.vector.memset(offA[:], 0.0)

            y_t = [ypool.tile([128, LH], F32, tag=f"y{cc}", name=f"y{cc}")
                   for cc in range(8)]

            for h in range(2):
                T = apool.tile([128, LH], F32, tag="T")
                Tsq = apool.tile([128, LH], F32, tag="Tsq")
                nc.vector.memset(T[:], 0.0)
                nc.vector.memset(Tsq[:], 0.0)

                # ---- conv for all cc of this half ----
                for cc in range(8):
                    strip = spool.tile([128, LH + 8], F32R, tag="strip")
                    nc.sync.dma_start(
                        strip[:], qT[128 * cc:128 * (cc + 1),
                                     LH * h:LH * h + LH + 8])
                    dW_t = dwpool.tile([128, KK * 128], F32R, tag="dWcc")
                    nc.sync.dma_start(
                        dW_t[:], dW[:, KK * 128 * cc:KK * 128 * (cc + 1)])
                    ps = [pconv.tile([128, 512], F32, tag=f"pc{lc}", name=f"pc{lc}")
                          for lc in range(NLC)]
                    for j in range(KK):
                        wslice = dW_t[:, 128 * j:128 * (j + 1)]
                        for lc in range(NLC):
                            nc.tensor.matmul(
                                ps[lc][:], wslice,
                                strip[:, 512 * lc + j:512 * lc + j + 512],
                                start=(j == 0), stop=False)
                    for lc in range(NLC):
                        nc.tensor.matmul(
                            ps[lc][:], bdw_row[:, 128 * cc:128 * (cc + 1)],
                            onesR, start=False, stop=True)
                    # evict + accumulate stats inputs
                    for lc in range(NLC):
                        nc.scalar.copy(y_t[cc][:, 512 * lc:512 * (lc + 1)],
                                       ps[lc][:])
                    ysq = ysqp.tile([128, LH], F32, tag="ysq")
                    nc.scalar.square(ysq[:], y_t[cc][:])
                    nc.vector.tensor_tensor(T[:], T[:], y_t[cc][:], ALU.add)
                    nc.vector.tensor_tensor(Tsq[:], Tsq[:], ysq[:], ALU.add)

                # ---- stats + normalize + gelu + offset, per 512-chunk ----
                for lc in range(NLC):
                    sl = slice(512 * lc, 512 * (lc + 1))
                    s1 = pstat.tile([1, 512], F32, tag="s1")
                    s2 = pstat.tile([1, 512], F32, tag="s2")
                    nc.tensor.matmul(s1[:], oneC, T[:, sl].bitcast(F32),
                                     start=True, stop=True)
                    nc.tensor.matmul(s2[:], oneC, Tsq[:, sl].bitcast(F32),
                                     start=True, stop=True)
                    mu = rows.tile([1, 512], F32, tag="mu")
                    nc.vector.tensor_scalar(mu[:], s1[:], 1.0 / C, None, ALU.mult)
                    e2 = rows.tile([1, 512], F32, tag="e2")
                    nc.vector.tensor_scalar(e2[:], s2[:], 1.0 / C, None, ALU.mult)
                    var = rows.tile([1, 512], F32, tag="var")
                    nc.vector.tensor_tensor(var[:], mu[:], mu[:], ALU.mult)
                    nc.vector.tensor_tensor(var[:], e2[:], var[:], ALU.subtract)
                    sd = rows.tile([1, 512], F32, tag="sd")
                    nc.scalar.activation(sd[:], var[:], AF.Sqrt, bias=eps_ap)
                    rstd = rows.tile([1, 512], F32, tag="rstd")
                    nc.vector.reciprocal(rstd[:], sd[:])
                    brow = rows.tile([1, 512], F32, tag="brow")
                    nc.vector.tensor_tensor(brow[:], mu[:], rstd[:], ALU.mult)
                    nc.vector.tensor_scalar(brow[:], brow[:], -1.0, None, ALU.mult)
                    # broadcast rows to [128, 512] via K=1 f32 matmuls
                    A_ps = pbc.tile([128, 512], F32, tag="A")
                    nc.tensor.matmul(A_ps[:], oneF, rstd[:], start=True, stop=True)
                    B_ps = pbc.tile([128, 512], F32, tag="Bb")
                    nc.tensor.matmul(B_ps[:], oneF, brow[:], start=True, stop=True)

                    for cc in range(8):
                        z = flow.tile([128, 512], F32, tag="z")
                        nc.vector.tensor_tensor(z[:], y_t[cc][:, sl], A_ps[:],
                                                ALU.mult)
                        nc.vector.tensor_tensor(z[:], z[:], B_ps[:], ALU.add)
                        nc.vector.tensor_scalar(
                            z[:], z[:],
                            pF_t[:, PF_LNW + cc:PF_LNW + cc + 1].bitcast(F32),
                            pF_t[:, PF_LNB + cc:PF_LNB + cc + 1].bitcast(F32),
                            ALU.mult, ALU.add)
                        g = flow.tile([128, 512], F32, tag="g")
                        nc.scalar.activation(g[:], z[:], AF.Gelu)
                        om = flow.tile([128, 512], F32, tag="om")
                        w2 = pF_t[:, PF_W2 + 64 * cc:PF_W2 + 64 * cc + 64]
                        nc.vector.tensor_tensor(
                            om[:].rearrange("p (k j) -> p k j", j=64),
                            g[:].rearrange("p (k j) -> p k j", j=64),
                            w2.bitcast(F32)[:, None, :].to_broadcast((128, 8, 64)),
                            ALU.mult)
                        red = flow.tile([128, 8], F32, tag="red")
                        nc.vector.tensor_reduce(
                            red[:], om[:].rearrange("p (k j) -> p k j", j=64),
                            AX.X, ALU.add)
                        ko = 32 * h + 8 * lc
                        nc.vector.tensor_tensor(offA[:, ko:ko + 8],
                                                offA[:, ko:ko + 8], red[:],
                                                ALU.add)

            # ---- offset signs -> one-hot gather matrix P2 [64, 128] ----
            offp = pstat.tile([1, 64], F32, tag="s1")
            nc.tensor.matmul(offp[:], oneC, offA[:].bitcast(F32),
                             start=True, stop=True)
            sgn = rows.tile([1, 64], F32, tag="sgn")
            nc.scalar.sign(sgn[:], offp[:])
            posd = rows.tile([1, 128], F32, tag="posd")
            # pos = k + 0.5 + 0.4*sign(off); duplicate into both 64-halves
            for lsb in range(2):
                nc.vector.scalar_tensor_tensor(
                    posd[:, 64 * lsb:64 * lsb + 64], sgn[:], 0.4,
                    pF_t[0:1, PF_KV:PF_KV + 64].bitcast(F32),
                    ALU.mult, ALU.add)
            nc.vector.tensor_scalar(posd[:], posd[:], 63.49, None, ALU.min)
            pos_ps = pbc.tile([64, 128], F32, tag="A")
            nc.tensor.matmul(pos_ps[:], oneF[:, 0:64], posd[:],
                             start=True, stop=True)
            dmat = scr.tile([64, 128], F32, tag="dmat")
            nc.vector.tensor_tensor(
                dmat[:], pF_t[0:64, PF_G2:PF_G2 + 128].bitcast(F32),
                pos_ps[:], ALU.subtract)
            nc.scalar.activation(dmat[:], dmat[:], AF.Abs)
            P2 = scr.tile([64, 128], F32, tag="P2")
            nc.vector.tensor_scalar(P2[:], dmat[:], 0.5, None, ALU.is_lt)

            # ---- gather: T2[64*lsb + k, u] = t_s[2u + lsb, idx[k]] ----
            T2_ps = pbc.tile([128, 512], F32, tag="Bb")
            nc.tensor.matmul(T2_ps[0:64, :], P2[:, 0:64],
                             tsT_t[:, 0:512], start=True, stop=True)
            nc.tensor.matmul(T2_ps[64:128, :], P2[:, 64:128],
                             tsT_t[:, 512:1024], start=True, stop=True,
                             tile_position=(0, 64))
            T2 = scr.tile([128, 512], F32, tag="T2")
            nc.scalar.copy(T2[:], T2_ps[:])

            # ---- proj: out[n,o] = sum_m x_sT[m,n] wqT[m,o] + bq[o] ----
            out_sb = scr.tile([64, C], F32, tag="outsb")
            wqr = wqT.rearrange("(t p) o -> p t o", p=128)
            for oh in range(2):
                wq_t = wqpool.tile([128, 8, 512], F32, tag="wqh")
                nc.sync.dma_start(wq_t[:], wqr[:, :, 512 * oh:512 * (oh + 1)])
                po = pconv.tile([64, 512], F32, tag="pc0")
                for t in range(8):
                    nc.tensor.matmul(po[:], T2[:, t::8],
                                     wq_t[:, t, :],
                                     start=(t == 0), stop=False)
                nc.tensor.matmul(po[:], oneF[:, 0:64],
                                 pF_t[0:1, PF_BQ + 512 * oh:PF_BQ + 512 * (oh + 1)]
                                 .bitcast(F32),
                                 start=False, stop=True)
                nc.scalar.copy(out_sb[:, 512 * oh:512 * (oh + 1)], po[:])
            nc.sync.dma_start(out[:, :], out_sb[:])

    nc.finalize()
    return nc


_NC_CACHE = {}


def _get_nc():
    if "nc" not in _NC_CACHE:
        nc = bacc.Bacc("TRN2", target_bir_lowering=False, debug=False,
                       num_devices=8)
        _NC_CACHE["nc"] = _build(nc)
    return _NC_CACHE["nc"]


def _host_prep(q, tokens, w_dw, b_dw, ln_w, ln_b, w_off, wq, bq):
    """Build per-core input maps."""
    q = np.ascontiguousarray(q, dtype=np.float32)
    tokens = np.ascontiguousarray(tokens, dtype=np.float32)

    # shared consts
    dW = np.zeros((128, 8 * KK * 128), dtype=np.float32)
    ar = np.arange(128)
    for ccx in range(8):
        for j in range(KK):
            dW[ar, KK * 128 * ccx + 128 * j + ar] = w_dw[128 * ccx + ar, 0, j]

    pR = np.zeros((1, NR), dtype=np.float32)
    pR[0, PR_ONE:PR_ONE + 512] = 1.0
    pR[0, PR_BDW:PR_BDW + C] = b_dw

    pF = np.zeros((128, NF), dtype=np.float32)
    pF[0, PF_BQ:PF_BQ + C] = bq
    pF[0, PF_KV:PF_KV + 64] = np.arange(64) + 0.5
    pF[0, PF_EPS] = EPS
    pF[0, PF_ONE:PF_ONE + 128] = 1.0
    pF[:, PF_ONEC] = 1.0
    pF[:, PF_LNW:PF_LNW + 8] = ln_w.reshape(8, 128).T
    pF[:, PF_LNB:PF_LNB + 8] = ln_b.reshape(8, 128).T
    pF[:, PF_W2:PF_W2 + 512] = w_off[0].reshape(8, 128, 64).transpose(
        1, 0, 2).reshape(128, 512)
    pF[0:64, PF_G2:PF_G2 + 128] = np.tile(
        np.arange(64, dtype=np.float32)[:, None], (1, 128))

    wqT = np.ascontiguousarray(wq.T)

    in_maps = []
    for b in range(B):
        qTb = np.zeros((C, L + 8), dtype=np.float32)
        qTb[:, 4:4 + L] = q[b, CLS:, :].T
        flat = tokens[b].reshape(-1)[C:]
        t_s = flat.reshape(C, L)[:, ::65]            # [C, 64]
        tsT = np.empty((64, 1024), dtype=np.float32)
        tsT[:, 0:512] = t_s[0::2, :].T               # even c
        tsT[:, 512:1024] = t_s[1::2, :].T            # odd c
        in_maps.append(dict(qT=qTb, dW=dW, pR=pR, pF=pF,
                            tsT=np.ascontiguousarray(tsT), wqT=wqT))
    return in_maps


def kernel(q, tokens, w_dw, b_dw, ln_w, ln_b, w_off, wq, bq, _trace=False):
    nc = _get_nc()
    in_maps = _host_prep(np.asarray(q), np.asarray(tokens), np.asarray(w_dw),
                         np.asarray(b_dw), np.asarray(ln_w), np.asarray(ln_b),
                         np.asarray(w_off), np.asarray(wq), np.asarray(bq))
    last_err = None
    for attempt in range(3):
        try:
            res = run_bass_kernel_spmd(nc, in_maps, core_ids=list(range(B)),
                                       trace=_trace)
            break
        except Exception as e:  # transient NRT/device hiccups: retry
            last_err = e
            import time as _time
            _time.sleep(2.0)
    else:
        raise last_err
    outs = np.stack([res.results[b]["out"] for b in range(B)])  # [8, 64, 1024]
    if _trace:
        kernel._last_result = res
    return outs.reshape(B * 16, 64, LK).astype(np.float32)


# revision 15
# speedup vs baseline: 1.0211x; 1.0023x over previous
"""Trainium2 Bass kernel for nn_DeformedAgent (sparse_attention).

Self-contained: kernel(**inputs) -> np.ndarray [128, 64, 64].

Math (validated against the neuron-executed reference):
  - The deformable gather index is idx[b,k] = clip(k + (offset_raw[b,k] > 0), <=63)
    because the device float->int cast rounds to nearest (ref points sit at
    k+0.5, tanh-bounded offsets |d|<1/63 never cross the next integer).
  - offset_raw = conv1d(w_off) o gelu o channelLN o depthwise-conv9(qT)+b.
    Only its SIGN is consumed, but it must be f32-accurate (min margin 3.5e-3).
  - The align_corners interp is an exact subsample t_s = tokens'[:, ::65]
    (src positions k*65 are integral), and the torch-style raw reshapes make
    the output out[b] = x_s @ wq.T + bq with x_s flat-aliasing t_s.

Per-core (8 cores, data-parallel over batch):
  phase 1: depthwise conv as 9 diag-matmuls + K=1 bias matmul (f32r),
           channel-LN via ones-matmul partition sums, ACT gelu,
           offset contraction on DVE + window reduce.
  phase 2: build one-hot gather matrix from sign(offset), apply as matmul,
           proj matmul (f32) + bias via K=1 matmul.
"""
import numpy as np

import concourse.bass as bass
import concourse.mybir as mybir
import concourse.tile as tile
from concourse import bacc
from concourse.bass_utils import run_bass_kernel_spmd

F32 = mybir.dt.float32
F32R = mybir.dt.float32r
AF = mybir.ActivationFunctionType
ALU = mybir.AluOpType
AX = mybir.AxisListType

B, L, C, CLS, LK, KK, AGENT = 8, 4096, 1024, 1, 64, 9, 64
EPS = 1e-5
LH = L // 2          # l per half = 2048
NLC = LH // 512      # 512-chunks per half = 4

# ---- packF (f32 consts) column layout, [128, NF] ----
PF_BQ = 0        # [1,1024] row0
PF_KV = 1024     # [1,64] row0: k+0.5
PF_EPS = 1088    # [1,1] row0
PF_ONE = 1096    # [1,128] row0 ones  (K=1 lhsT / bcast lhsT)
PF_ONEC = 1224   # [128,1] ones column (partition-sum lhsT)
PF_LNW = 1225    # [128,8]
PF_LNB = 1233    # [128,8]
PF_W2 = 1241     # [128, 8*64] w_off arranged
PF_G2 = 1753     # [64,128] rows0:64: G2[kk',p]=kk'
NF = 1881

# ---- packR (f32r consts) ----
PR_ONE = 0       # [1,512] row0 ones
PR_BDW = 512     # [1,1024] row0 b_dw
NR = 1536


def _build(nc):
    qT = nc.dram_tensor("qT", [C, L + 8], F32R, kind="ExternalInput")
    dW = nc.dram_tensor("dW", [128, 8 * KK * 128], F32R, kind="ExternalInput")
    pR = nc.dram_tensor("pR", [1, NR], F32R, kind="ExternalInput")
    pF = nc.dram_tensor("pF", [128, NF], F32, kind="ExternalInput")
    tsT = nc.dram_tensor("tsT", [64, 1024], F32, kind="ExternalInput")  # even|odd
    wqT = nc.dram_tensor("wqT", [C, C], F32, kind="ExternalInput")
    out = nc.dram_tensor("out", [64, C], F32, kind="ExternalOutput")

    with tile.TileContext(nc) as tc:
        with (
            tc.tile_pool(name="consts", bufs=1) as cpool,
            tc.tile_pool(name="strips", bufs=2) as spool,
            tc.tile_pool(name="dwpool", bufs=2) as dwpool,
            tc.tile_pool(name="ybuf", bufs=1) as ypool,
            tc.tile_pool(name="acc", bufs=1) as apool,
            tc.tile_pool(name="flow", bufs=4) as flow,
            tc.tile_pool(name="scratch", bufs=1) as scr,
            tc.tile_pool(name="ysqp", bufs=1) as ysqp,
            tc.tile_pool(name="rows", bufs=1) as rows,
            tc.tile_pool(name="wq", bufs=1) as wqpool,
            tc.tile_pool(name="pconv", bufs=1, space="PSUM") as pconv,
            tc.tile_pool(name="pstat", bufs=1, space="PSUM") as pstat,
            tc.tile_pool(name="pbc", bufs=1, space="PSUM") as pbc,
        ):
            pR_t = cpool.tile([1, NR], F32R, tag="pR")
            nc.sync.dma_start(pR_t[:], pR[:, :])
            pF_t = cpool.tile([128, NF], F32, tag="pF")
            nc.sync.dma_start(pF_t[:], pF[:, :])
            tsT_t = cpool.tile([64, 1024], F32, tag="tsT")
            nc.sync.dma_start(tsT_t[:], tsT[:, :])

            onesR = pR_t[0:1, PR_ONE:PR_ONE + 512]
            bdw_row = pR_t[0:1, PR_BDW:PR_BDW + C]
            oneF = pF_t[0:1, PF_ONE:PF_ONE + 128].bitcast(F32)
            oneC = pF_t[:, PF_ONEC:PF_ONEC + 1].bitcast(F32)
            eps_ap = pF_t[0:1, PF_EPS:PF_EPS + 1].bitcast(F32)

            # offset accumulator [128, 64] f32
            offA = apool.tile([128, LK], F32, tag="offA")
            nc.vector.memset(offA[:], 0.0)

            y_t = [ypool.tile([128, LH], F32, tag=f"y{cc}", name=f"y{cc}")
                   for cc in range(8)]

            for h in range(2):
                T = apool.tile([128, LH], F32, tag="T")
                Tsq = apool.tile([128, LH], F32, tag="Tsq")
                nc.vector.memset(T[:], 0.0)
                nc.vector.memset(Tsq[:], 0.0)

                # ---- conv for all cc of this half ----
                for cc in range(8):
                    strip = spool.tile([128, LH + 8], F32R, tag="strip")
                    nc.sync.dma_start(
                        strip[:], qT[128 * cc:128 * (cc + 1),
                                     LH * h:LH * h + LH + 8])
                    dW_t = dwpool.tile([128, KK * 128], F32R, tag="dWcc")
                    nc.sync.dma_start(
                        dW_t[:], dW[:, KK * 128 * cc:KK * 128 * (cc + 1)])
                    ps = [pconv.tile([128, 512], F32, tag=f"pc{lc}", name=f"pc{lc}")
                          for lc in range(NLC)]
                    for j in range(KK):
                        wslice = dW_t[:, 128 * j:128 * (j + 1)]
                        for lc in range(NLC):
                            nc.tensor.matmul(
                                ps[lc][:], wslice,
                                strip[:, 512 * lc + j:512 * lc + j + 512],
                                start=(j == 0), stop=False)
                    for lc in range(NLC):
                        nc.tensor.matmul(
                            ps[lc][:], bdw_row[:, 128 * cc:128 * (cc + 1)],
                            onesR, start=False, stop=True)
                    # evict + accumulate stats inputs
                    for lc in range(NLC):
                        nc.scalar.copy(y_t[cc][:, 512 * lc:512 * (lc + 1)],
                                       ps[lc][:])
                    ysq = ysqp.tile([128, LH], F32, tag="ysq")
                    nc.scalar.square(ysq[:], y_t[cc][:])
                    nc.vector.tensor_tensor(T[:], T[:], y_t[cc][:], ALU.add)
                    nc.vector.tensor_tensor(Tsq[:], Tsq[:], ysq[:], ALU.add)

                # ---- stats + normalize + gelu + offset, per 512-chunk ----
                for lc in range(NLC):
                    sl = slice(512 * lc, 512 * (lc + 1))
                    s1 = pstat.tile([1, 512], F32, tag="s1")
                    s2 = pstat.tile([1, 512], F32, tag="s2")
                    nc.tensor.matmul(s1[:], oneC, T[:, sl].bitcast(F32),
                                     start=True, stop=True)
                    nc.tensor.matmul(s2[:], oneC, Tsq[:, sl].bitcast(F32),
                                     start=True, stop=True)
                    mu = rows.tile([1, 512], F32, tag="mu")
                    nc.vector.tensor_scalar(mu[:], s1[:], 1.0 / C, None, ALU.mult)
                    e2 = rows.tile([1, 512], F32, tag="e2")
                    nc.vector.tensor_scalar(e2[:], s2[:], 1.0 / C, None, ALU.mult)
                    var = rows.tile([1, 512], F32, tag="var")
                    nc.vector.tensor_tensor(var[:], mu[:], mu[:], ALU.mult)
                    nc.vector.tensor_tensor(var[:], e2[:], var[:], ALU.subtract)
                    sd = rows.tile([1, 512], F32, tag="sd")
                    nc.scalar.activation(sd[:], var[:], AF.Sqrt, bias=eps_ap)
                    rstd = rows.tile([1, 512], F32, tag="rstd")
                    nc.vector.reciprocal(rstd[:], sd[:])
                    brow = rows.tile([1, 512], F32, tag="brow")
                    nc.vector.tensor_tensor(brow[:], mu[:], rstd[:], ALU.mult)
                    nc.vector.tensor_scalar(brow[:], brow[:], -1.0, None, ALU.mult)
                    # broadcast rows to [128, 512] via K=1 f32 matmuls
                    A_ps = pbc.tile([128, 512], F32, tag="A")
                    nc.tensor.matmul(A_ps[:], oneF, rstd[:], start=True, stop=True)
                    B_ps = pbc.tile([128, 512], F32, tag="Bb")
                    nc.tensor.matmul(B_ps[:], oneF, brow[:], start=True, stop=True)

                    for cc in range(8):
                        z = flow.tile([128, 512], F32, tag="z")
                        nc.vector.tensor_tensor(z[:], y_t[cc][:, sl], A_ps[:],
                                                ALU.mult)
                        nc.vector.tensor_tensor(z[:], z[:], B_ps[:], ALU.add)
                        g = flow.tile([128, 512], F32, tag="g")
                        nc.scalar.activation(
                            g[:], z[:], AF.Gelu,
                            bias=pF_t[:, PF_LNB + cc:PF_LNB + cc + 1].bitcast(F32),
                            scale=pF_t[:, PF_LNW + cc:PF_LNW + cc + 1].bitcast(F32))
                        om = flow.tile([128, 512], F32, tag="om")
                        w2 = pF_t[:, PF_W2 + 64 * cc:PF_W2 + 64 * cc + 64]
                        nc.vector.tensor_tensor(
                            om[:].rearrange("p (k j) -> p k j", j=64),
                            g[:].rearrange("p (k j) -> p k j", j=64),
                            w2.bitcast(F32)[:, None, :].to_broadcast((128, 8, 64)),
                            ALU.mult)
                        red = flow.tile([128, 8], F32, tag="red")
                        nc.vector.tensor_reduce(
                            red[:], om[:].rearrange("p (k j) -> p k j", j=64),
                            AX.X, ALU.add)
                        ko = 32 * h + 8 * lc
                        nc.vector.tensor_tensor(offA[:, ko:ko + 8],
                                                offA[:, ko:ko + 8], red[:],
                                                ALU.add)

            # ---- offset signs -> one-hot gather matrix P2 [64, 128] ----
            offp = pstat.tile([1, 64], F32, tag="s1")
            nc.tensor.matmul(offp[:], oneC, offA[:].bitcast(F32),
                             start=True, stop=True)
            sgn = rows.tile([1, 64], F32, tag="sgn")
            nc.scalar.sign(sgn[:], offp[:])
            posd = rows.tile([1, 128], F32, tag="posd")
            # pos = k + 0.5 + 0.4*sign(off); duplicate into both 64-halves
            for lsb in range(2):
                nc.vector.scalar_tensor_tensor(
                    posd[:, 64 * lsb:64 * lsb + 64], sgn[:], 0.4,
                    pF_t[0:1, PF_KV:PF_KV + 64].bitcast(F32),
                    ALU.mult, ALU.add)
            nc.vector.tensor_scalar(posd[:], posd[:], 63.49, None, ALU.min)
            pos_ps = pbc.tile([64, 128], F32, tag="A")
            nc.tensor.matmul(pos_ps[:], oneF[:, 0:64], posd[:],
                             start=True, stop=True)
            dmat = scr.tile([64, 128], F32, tag="dmat")
            nc.vector.tensor_tensor(
                dmat[:], pF_t[0:64, PF_G2:PF_G2 + 128].bitcast(F32),
                pos_ps[:], ALU.subtract)
            nc.scalar.activation(dmat[:], dmat[:], AF.Abs)
            P2 = scr.tile([64, 128], F32, tag="P2")
            nc.vector.tensor_scalar(P2[:], dmat[:], 0.5, None, ALU.is_lt)

            # ---- gather: T2[64*lsb + k, u] = t_s[2u + lsb, idx[k]] ----
            T2_ps = pbc.tile([128, 512], F32, tag="Bb")
            nc.tensor.matmul(T2_ps[0:64, :], P2[:, 0:64],
                             tsT_t[:, 0:512], start=True, stop=True)
            nc.tensor.matmul(T2_ps[64:128, :], P2[:, 64:128],
                             tsT_t[:, 512:1024], start=True, stop=True,
                             tile_position=(0, 64))
            T2 = scr.tile([128, 512], F32, tag="T2")
            nc.scalar.copy(T2[:], T2_ps[:])

            # ---- proj: out[n,o] = sum_m x_sT[m,n] wqT[m,o] + bq[o] ----
            out_sb = scr.tile([64, C], F32, tag="outsb")
            wqr = wqT.rearrange("(t p) o -> p t o", p=128)
            for oh in range(2):
                wq_t = wqpool.tile([128, 8, 512], F32, tag="wqh")
                nc.sync.dma_start(wq_t[:], wqr[:, :, 512 * oh:512 * (oh + 1)])
                po = pconv.tile([64, 512], F32, tag="pc0")
                for t in range(8):
                    nc.tensor.matmul(po[:], T2[:, t::8],
                                     wq_t[:, t, :],
                                     start=(t == 0), stop=False)
                nc.tensor.matmul(po[:], oneF[:, 0:64],
                                 pF_t[0:1, PF_BQ + 512 * oh:PF_BQ + 512 * (oh + 1)]
                                 .bitcast(F32),
                                 start=False, stop=True)
                nc.scalar.copy(out_sb[:, 512 * oh:512 * (oh + 1)], po[:])
            nc.sync.dma_start(out[:, :], out_sb[:])

    nc.finalize()
    return nc


_NC_CACHE = {}


def _get_nc():
    if "nc" not in _NC_CACHE:
        nc = bacc.Bacc("TRN2", target_bir_lowering=False, debug=False,
                       num_devices=8)
        _NC_CACHE["nc"] = _build(nc)
    return _NC_CACHE["nc"]


def _host_prep(q, tokens, w_dw, b_dw, ln_w, ln_b, w_off, wq, bq):
    """Build per-core input maps."""
    q = np.ascontiguousarray(q, dtype=np.float32)
    tokens = np.ascontiguousarray(tokens, dtype=np.float32)

    # shared consts
    dW = np.zeros((128, 8 * KK * 128), dtype=np.float32)
    ar = np.arange(128)
    for ccx in range(8):
        for j in range(KK):
            dW[ar, KK * 128 * ccx + 128 * j + ar] = w_dw[128 * ccx + ar, 0, j]

    pR = np.zeros((1, NR), dtype=np.float32)
    pR[0, PR_ONE:PR_ONE + 512] = 1.0
    pR[0, PR_BDW:PR_BDW + C] = b_dw

    pF = np.zeros((128, NF), dtype=np.float32)
    pF[0, PF_BQ:PF_BQ + C] = bq
    pF[0, PF_KV:PF_KV + 64] = np.arange(64) + 0.5
    pF[0, PF_EPS] = EPS
    pF[0, PF_ONE:PF_ONE + 128] = 1.0
    pF[:, PF_ONEC] = 1.0
    pF[:, PF_LNW:PF_LNW + 8] = ln_w.reshape(8, 128).T
    pF[:, PF_LNB:PF_LNB + 8] = ln_b.reshape(8, 128).T
    pF[:, PF_W2:PF_W2 + 512] = w_off[0].reshape(8, 128, 64).transpose(
        1, 0, 2).reshape(128, 512)
    pF[0:64, PF_G2:PF_G2 + 128] = np.tile(
        np.arange(64, dtype=np.float32)[:, None], (1, 128))

    wqT = np.ascontiguousarray(wq.T)

    in_maps = []
    for b in range(B):
        qTb = np.zeros((C, L + 8), dtype=np.float32)
        qTb[:, 4:4 + L] = q[b, CLS:, :].T
        flat = tokens[b].reshape(-1)[C:]
        t_s = flat.reshape(C, L)[:, ::65]            # [C, 64]
        tsT = np.empty((64, 1024), dtype=np.float32)
        tsT[:, 0:512] = t_s[0::2, :].T               # even c
        tsT[:, 512:1024] = t_s[1::2, :].T            # odd c
        in_maps.append(dict(qT=qTb, dW=dW, pR=pR, pF=pF,
                            tsT=np.ascontiguousarray(tsT), wqT=wqT))
    return in_maps


def kernel(q, tokens, w_dw, b_dw, ln_w, ln_b, w_off, wq, bq, _trace=False):
    nc = _get_nc()
    in_maps = _host_prep(np.asarray(q), np.asarray(tokens), np.asarray(w_dw),
                         np.asarray(b_dw), np.asarray(ln_w), np.asarray(ln_b),
                         np.asarray(w_off), np.asarray(wq), np.asarray(bq))
    last_err = None
    for attempt in range(3):
        try:
            res = run_bass_kernel_spmd(nc, in_maps, core_ids=list(range(B)),
                                       trace=_trace)
            break
        except Exception as e:  # transient NRT/device hiccups: retry
            last_err = e
            import time as _time
            _time.sleep(2.0)
    else:
        raise last_err
    outs = np.stack([res.results[b]["out"] for b in range(B)])  # [8, 64, 1024]
    if _trace:
        kernel._last_result = res
    return outs.reshape(B * 16, 64, LK).astype(np.float32)
